# revision 32
# baseline (speedup 1.0000x reference)
"""Trainium2 Bass kernel for nn_CondIndepenLoss (v3 — bf16 streams, host one-hots).

Computes, for B=65536 rows sharded 8192/core over 8 NeuronCores:
    jp   = softmax(joint_probs[:, :64])                      [B, 64]
    LS   = log(softmax(pred_probs, axis=2) + eps)            [3, B, 10]
    lp[b,c] = sum_d LS[d, b, valid_cp[c,d]]
    w[b] = exp(-0.5*(|Z_b|^2 + |X_b - Xhat_b|^2))
    vals[b] = jp[b,y] * w[b] * (log(jp[b,y]+eps) - lp[b,y]),  y = Y_valid[b]
    loss = |sum_b vals[b] * (y<64)| / count(y<64)

Design:
  - all big HBM streams are bf16 (host casts): [x|z] 640, xh 512,
    [jp|pp|ohj|ohp] 188 elems/row -> ~22 MB/core instead of 41 MB
  - softmax handled in log space: selected logit - ln(sum exp); the
    selection one-hots are built host-side (exact 0/1 in bf16) and ride
    the jq stream, so the select is one multiply + one grouped reduce
  - ssq = |dx|^2+|z|^2 per row: subtract grouped on DVE (2x bf16) with a
    GpSimd share; square+reduce split between a grouped DVE pair
    (tensor_tensor mult + grouped tensor_reduce) and ScalarE
    Square+accum_out slices (tensor_tensor_reduce crashes TRN2 firmware)
  - one-hot multiplies run on GpSimd
  - three DMA rings: sync (x|z), scalar (xh), gpsimd (jq + consts)
  - final pointwise math once over [128, 64] column buffers, PE reduces
    across partitions, host combines the 8 (sum, count) pairs
"""

import os
import sys

import numpy as np

for _p in ("/opt/trn_rl_repo",):
    if os.path.isdir(_p) and _p not in sys.path:
        sys.path.insert(0, _p)

from contextlib import ExitStack

import ml_dtypes

from concourse import bacc, bass, mybir, tile
from concourse.bass_utils import run_bass_kernel_spmd

M = 8                     # cores
B = 65536
BL = B // M               # 8192 rows per core
P = 128                   # SBUF partitions
XD, ZD, C, D, K = 512, 128, 64, 3, 10
XZ = XD + ZD              # 640
JQ = 2 * (C + D * K)      # 188  ([jp|pp|ohj|ohp] elems per row)
S = 8                     # rows per partition per iteration
NA = 8                    # iterations: 1024 rows each
RA = P * S                # rows per iteration (1024)
NT = NA * S               # 64 column slots total
F32 = mybir.dt.float32
BF16 = mybir.dt.bfloat16
FP8 = mybir.dt.float8e4

# per-iteration: how many of the 8 ssq slices go through the grouped DVE
# square+reduce pair (the rest run as ScalarE Square+accum_out slices).
# The last iteration leans on ACT so DVE finishes the tail sooner.
N_DVE_SQ = 4
DVE_SQ_SCHED = [4] * 7 + [2]       # optional explicit per-iter list overriding N_DVE_SQ
# of the 8 subtract slices per iteration: how many run on GpSimd
N_GPS_SUB = 2

_NC_CACHE = {}

_ACT_SET = "natural_log_exp_and_others"


def _pin_act_tables():
    """Make the table-load pass see only one usable activation set so the
    whole kernel shares a single ACT_TABLE_LOAD (Exp/Ln/Square all live in
    natural_log_exp_and_others)."""
    import concourse.bacc as bacc_mod
    from concourse.hw_specs import get_activation_tables

    real = get_activation_tables  # functools.cache'd original

    def patched(arch):
        tabs = real(arch)
        return {
            name: (funcs if name == _ACT_SET else set())
            for name, funcs in tabs.items()
        }

    bacc_mod.get_activation_tables = patched


def _build_nc():
    AluOp = mybir.AluOpType
    ACT = mybir.ActivationFunctionType
    AX = mybir.AxisListType

    _pin_act_tables()
    nc = bacc.Bacc("TRN2", target_bir_lowering=False, debug=False, num_devices=M)

    xz_d = nc.dram_tensor("xz", [BL, XZ], BF16, kind="ExternalInput")
    xh_d = nc.dram_tensor("xh", [BL, XD], BF16, kind="ExternalInput")
    jq_d = nc.dram_tensor("jq", [BL, JQ], FP8, kind="ExternalInput")
    cst_d = nc.dram_tensor("cst", [P, NT], BF16, kind="ExternalInput")
    out_d = nc.dram_tensor("out", [1, 2], F32, kind="ExternalOutput")

    with tile.TileContext(nc) as tc, ExitStack() as ctx:
        cpool = ctx.enter_context(tc.tile_pool(name="consts", bufs=1))
        apool = ctx.enter_context(tc.tile_pool(name="a", bufs=4))
        bpool = ctx.enter_context(tc.tile_pool(name="b", bufs=3))
        spool = ctx.enter_context(tc.tile_pool(name="s", bufs=2))
        accp = ctx.enter_context(tc.tile_pool(name="acc", bufs=1))
        psp = ctx.enter_context(
            tc.tile_pool(name="ps", bufs=1, space=bass.MemorySpace.PSUM)
        )

        ybuf = cpool.tile([P, NT], BF16)        # y at column slot t
        ones = cpool.tile([P, 1], F32)

        ssqb = accp.tile([P, NT], F32)          # |dx|^2 + |z|^2 per row
        sjpb = accp.tile([P, NT], F32)          # sum_c exp(joint logit)
        jselb = accp.tile([P, NT], F32)         # joint logit at y
        s3b = accp.tile([P, NT, D], F32)        # per-dim sum_k exp(pred logit)
        nlselb = accp.tile([P, NT], F32)        # -sum_d pred logit at valid_cp[y]

        fb = accp.tile([P, 2, NT], F32)
        nc.gpsimd.dma_start(out=ybuf[:], in_=cst_d[:, 0:NT])
        nc.vector.memset(ones[:], 1.0)
        nc.vector.tensor_scalar(
            out=fb[:, 1, :], in0=ybuf[:], scalar1=float(C), scalar2=None,
            op0=AluOp.is_lt,
        )

        def emit_iter(i, ndve):
            r = slice(i * RA, (i + 1) * RA)
            cols = slice(i * S, (i + 1) * S)
            first = i == 0
            ct = apool.tile([P, S, XZ], BF16, tag="ct")
            xh = apool.tile([P, S, XD], BF16, tag="xh")
            jt = bpool.tile([P, S, JQ], FP8, tag="jt")
            hs = S // 2
            xzv = xz_d[r, :].rearrange("(p s) d -> p s d", s=S)
            xhv = xh_d[r, :].rearrange("(p s) d -> p s d", s=S)
            nc.sync.dma_start(out=ct[:, 0:hs, :], in_=xzv[:, 0:hs, :])
            nc.sync.dma_start(out=ct[:, hs:S, :], in_=xzv[:, hs:S, :])
            if first:
                nc.scalar.dma_start(out=xh[:, 0:hs, :], in_=xhv[:, 0:hs, :])
                nc.scalar.dma_start(out=xh[:, hs:S, :], in_=xhv[:, hs:S, :])
            else:
                nc.scalar.dma_start(out=xh[:], in_=xhv)
            nc.gpsimd.dma_start(
                out=jt[:], in_=jq_d[r, :].rearrange("(p s) d -> p s d", s=S)
            )

            # --- phase A: dx = x - xh (in place), then ssq per row ---
            nds = S - N_GPS_SUB  # slices subtracted on DVE
            if first:
                # halved ops so compute starts as soon as half the tile lands
                hd = nds // 2
                nc.vector.tensor_tensor(
                    out=ct[:, 0:hd, 0:XD], in0=ct[:, 0:hd, 0:XD],
                    in1=xh[:, 0:hd, :], op=AluOp.subtract,
                )
                nc.vector.tensor_tensor(
                    out=ct[:, hd:nds, 0:XD], in0=ct[:, hd:nds, 0:XD],
                    in1=xh[:, hd:nds, :], op=AluOp.subtract,
                )
            else:
                nc.vector.tensor_tensor(
                    out=ct[:, 0:nds, 0:XD], in0=ct[:, 0:nds, 0:XD],
                    in1=xh[:, 0:nds, :], op=AluOp.subtract,
                )
            nc.gpsimd.tensor_tensor(
                out=ct[:, nds:S, 0:XD], in0=ct[:, nds:S, 0:XD],
                in1=xh[:, nds:S, :], op=AluOp.subtract,
            )
            if ndve > 0:
                sq = spool.tile([P, ndve, XZ], BF16, tag="sq")
                nc.vector.tensor_tensor(
                    out=sq[:], in0=ct[:, 0:ndve, :], in1=ct[:, 0:ndve, :],
                    op=AluOp.mult,
                )
                nc.vector.tensor_reduce(
                    out=ssqb[:, i * S:i * S + ndve], in_=sq[:],
                    axis=AX.X, op=AluOp.add,
                )
            for s in range(ndve, S):
                t = i * S + s
                o = spool.tile([P, XZ], BF16, tag="acto")
                nc.scalar.activation(
                    out=o[:], in_=ct[:, s, :], func=ACT.Square,
                    accum_out=ssqb[:, t:t + 1],
                )

            # --- phase B: joint + pred log-softmax pieces ---
            # jq row: [jl (64) | pl (30) | ohj (64) | -ohp (30)]
            HQ = C + D * K                                      # 94
            ef = bpool.tile([P, S, HQ], F32, tag="ef")
            nc.scalar.activation(out=ef[:], in_=jt[:, :, 0:HQ], func=ACT.Exp)
            nc.vector.tensor_reduce(
                out=sjpb[:, cols], in_=ef[:, :, 0:C], axis=AX.X, op=AluOp.add
            )
            nc.vector.tensor_reduce(
                out=s3b[:, cols, :],
                in_=ef[:, :, C:HQ].rearrange("p s (d k) -> p s d k", k=K),
                axis=AX.X, op=AluOp.add,
            )
            # one-hot select with a single GpSimd multiply over [jl|pl]
            # (the pred one-hot is shipped negated, so the [64:94] sum
            # yields -sum_d lsel_d)
            ohm = bpool.tile([P, S, HQ], BF16, tag="ohm")
            nc.gpsimd.tensor_tensor(
                out=ohm[:], in0=jt[:, :, HQ:JQ], in1=jt[:, :, 0:HQ],
                op=AluOp.mult,
            )
            nc.vector.tensor_reduce(
                out=jselb[:, cols], in_=ohm[:, :, 0:C], axis=AX.X, op=AluOp.add
            )
            nc.vector.tensor_reduce(
                out=nlselb[:, cols], in_=ohm[:, :, C:HQ], axis=AX.X,
                op=AluOp.add,
            )

        sched = DVE_SQ_SCHED or [N_DVE_SQ] * NA
        for i in range(NA):
            emit_iter(i, sched[i])

        # --- epilogue over the whole core's 8192 rows ---
        lnsjp = accp.tile([P, NT], F32)
        s3p = accp.tile([P, NT], F32)
        lns3p = accp.tile([P, NT], F32)
        jd = accp.tile([P, NT], F32)
        t2 = accp.tile([P, NT], F32)
        diff = accp.tile([P, NT], F32)
        jps = accp.tile([P, NT], F32)
        wv = accp.tile([P, NT], F32)
        rr = accp.tile([P, 2], F32)
        ps = psp.tile([1, 2], F32)
        osb = accp.tile([1, 2], F32)

        nc.scalar.activation(out=lnsjp[:], in_=sjpb[:], func=ACT.Ln)
        nc.gpsimd.tensor_tensor(
            out=s3p[:], in0=s3b[:, :, 0], in1=s3b[:, :, 1], op=AluOp.mult
        )
        nc.gpsimd.tensor_tensor(
            out=s3p[:], in0=s3p[:], in1=s3b[:, :, 2], op=AluOp.mult
        )
        nc.scalar.activation(out=lns3p[:], in_=s3p[:], func=ACT.Ln)
        # lnjp = jsel - lnsjp ; lp = lsel - lns3p ; diff = lnjp - lp
        nc.vector.tensor_tensor(out=jd[:], in0=jselb[:], in1=lnsjp[:],
                                op=AluOp.subtract)
        nc.gpsimd.tensor_tensor(out=t2[:], in0=lns3p[:], in1=nlselb[:],
                                op=AluOp.add)
        nc.gpsimd.tensor_tensor(out=diff[:], in0=jd[:], in1=t2[:],
                                op=AluOp.add)
        nc.scalar.activation(out=jps[:], in_=jd[:], func=ACT.Exp)
        nc.scalar.activation(out=wv[:], in_=ssqb[:], func=ACT.Exp, scale=-0.5)
        nc.gpsimd.tensor_tensor(out=diff[:], in0=diff[:], in1=jps[:],
                                op=AluOp.mult)
        nc.gpsimd.tensor_tensor(out=diff[:], in0=diff[:], in1=wv[:],
                                op=AluOp.mult)
        nc.vector.tensor_tensor(out=fb[:, 0, :], in0=diff[:], in1=fb[:, 1, :],
                                op=AluOp.mult)
        nc.vector.tensor_reduce(out=rr[:], in_=fb[:], axis=AX.X, op=AluOp.add)
        nc.tensor.matmul(ps[:], ones[:], rr[:], start=True, stop=True)
        nc.vector.tensor_copy(out=osb[:], in_=ps[:])
        nc.sync.dma_start(out=out_d[:], in_=osb[:])

    nc.compile()
    return nc


def _get_nc():
    if "nc" not in _NC_CACHE:
        _NC_CACHE["nc"] = _build_nc()
    return _NC_CACHE["nc"]


def _col_layout(arr):
    """[BL, ...] per-core rows -> [P, NT, ...] SBUF column layout where row
    i*RA + p*S + s lands at [p, i*S + s]."""
    tail = arr.shape[1:]
    a = arr.reshape(NA, P, S, *tail)          # [i, p, s, ...]
    a = np.moveaxis(a, 1, 0)                  # [p, i, s, ...]
    return np.ascontiguousarray(a.reshape(P, NT, *tail))


def _prep_in_maps(inputs):
    bf16 = ml_dtypes.bfloat16
    X = np.asarray(inputs["X"], dtype=np.float32)
    Z = np.asarray(inputs["Z"], dtype=np.float32)
    XZc = np.concatenate([X, Z], axis=1).astype(bf16)
    Xh = np.asarray(inputs["X_hat"], dtype=np.float32).astype(bf16)
    jp64 = np.asarray(inputs["joint_probs"], dtype=np.float32)[:, :C]
    ppf = (
        np.asarray(inputs["pred_probs"], dtype=np.float32)
        .transpose(1, 0, 2)
        .reshape(B, D * K)
    )
    y = np.asarray(inputs["Y_valid"])
    vcp = np.asarray(inputs["valid_cp"])
    y_safe = np.where(y < C, y, 0).astype(np.int64)
    v3 = vcp[y_safe]                          # [B, 3]
    bidx = np.arange(B)
    ohj = np.zeros((B, C), np.float32)
    ohj[bidx, y_safe] = 1.0
    ohp = np.zeros((B, D, K), np.float32)
    for d in range(D):
        ohp[bidx, d, v3[:, d]] = -1.0          # negated: [64:94] sum = -lsel
    JQc = np.concatenate(
        [jp64, ppf, ohj, ohp.reshape(B, D * K)], axis=1
    ).astype(ml_dtypes.float8_e4m3)
    y32 = y.astype(np.float32)

    in_maps = []
    for m in range(M):
        s = slice(m * BL, (m + 1) * BL)
        in_maps.append(
            {
                "xz": np.ascontiguousarray(XZc[s]),
                "xh": np.ascontiguousarray(Xh[s]),
                "jq": np.ascontiguousarray(JQc[s]),
                "cst": _col_layout(y32[s]).astype(bf16),
            }
        )
    return in_maps


def _combine(results):
    tot = 0.0
    cnt = 0.0
    for r in results:
        o = np.asarray(r["out"], dtype=np.float64)
        tot += float(o[0, 0])
        cnt += float(o[0, 1])
    loss = abs(tot)
    val = loss / cnt if cnt > 0 else loss
    return np.float32(val)


def run(inputs, trace=False, **kwargs):
    """Build (cached), run on the 8 NeuronCores, return (value, BassKernelResults)."""
    nc = _get_nc()
    in_maps = _prep_in_maps(inputs)
    res = run_bass_kernel_spmd(nc, in_maps, list(range(M)), trace=trace, **kwargs)
    return _combine(res.results), res


def kernel(**inputs):
    val, _ = run(inputs, trace=False)
    return val


# revision 34
# speedup vs baseline: 1.0475x; 1.0475x over previous
"""Trainium2 Bass kernel for nn_CondIndepenLoss (v3 — bf16 streams, host one-hots).

Computes, for B=65536 rows sharded 8192/core over 8 NeuronCores:
    jp   = softmax(joint_probs[:, :64])                      [B, 64]
    LS   = log(softmax(pred_probs, axis=2) + eps)            [3, B, 10]
    lp[b,c] = sum_d LS[d, b, valid_cp[c,d]]
    w[b] = exp(-0.5*(|Z_b|^2 + |X_b - Xhat_b|^2))
    vals[b] = jp[b,y] * w[b] * (log(jp[b,y]+eps) - lp[b,y]),  y = Y_valid[b]
    loss = |sum_b vals[b] * (y<64)| / count(y<64)

Design:
  - all big HBM streams are bf16 (host casts): [x|z] 640, xh 512,
    [jp|pp|ohj|ohp] 188 elems/row -> ~22 MB/core instead of 41 MB
  - softmax handled in log space: selected logit - ln(sum exp); the
    selection one-hots are built host-side (exact 0/1 in bf16) and ride
    the jq stream, so the select is one multiply + one grouped reduce
  - ssq = |dx|^2+|z|^2 per row: subtract grouped on DVE (2x bf16) with a
    GpSimd share; square+reduce split between a grouped DVE pair
    (tensor_tensor mult + grouped tensor_reduce) and ScalarE
    Square+accum_out slices (tensor_tensor_reduce crashes TRN2 firmware)
  - one-hot multiplies run on GpSimd
  - three DMA rings: sync (x|z), scalar (xh), gpsimd (jq + consts)
  - final pointwise math once over [128, 64] column buffers, PE reduces
    across partitions, host combines the 8 (sum, count) pairs
"""

import os
import sys

import numpy as np

for _p in ("/opt/trn_rl_repo",):
    if os.path.isdir(_p) and _p not in sys.path:
        sys.path.insert(0, _p)

from contextlib import ExitStack

import ml_dtypes

from concourse import bacc, bass, mybir, tile
from concourse.bass_utils import run_bass_kernel_spmd

M = 8                     # cores
B = 65536
BL = B // M               # 8192 rows per core
P = 128                   # SBUF partitions
XD, ZD, C, D, K = 512, 128, 64, 3, 10
XZ = XD + ZD              # 640
JQ = 2 * (C + D * K)      # 188  ([jp|pp|ohj|ohp] elems per row)
S = 8                     # rows per partition per iteration
NA = 8                    # iterations: 1024 rows each
RA = P * S                # rows per iteration (1024)
NT = NA * S               # 64 column slots total
F32 = mybir.dt.float32
BF16 = mybir.dt.bfloat16
FP8 = mybir.dt.float8e4

# per-iteration: how many of the 8 ssq slices go through the grouped DVE
# square+reduce pair (the rest run as ScalarE Square+accum_out slices).
# The last iteration leans on ACT so DVE finishes the tail sooner.
N_DVE_SQ = 4
DVE_SQ_SCHED = [4] * 7 + [2]       # optional explicit per-iter list overriding N_DVE_SQ
# of the 8 subtract slices per iteration: how many run on GpSimd
N_GPS_SUB = 2

_NC_CACHE = {}

_ACT_SET = "natural_log_exp_and_others"


def _pin_act_tables():
    """Make the table-load pass see only one usable activation set so the
    whole kernel shares a single ACT_TABLE_LOAD (Exp/Ln/Square all live in
    natural_log_exp_and_others)."""
    import concourse.bacc as bacc_mod
    from concourse.hw_specs import get_activation_tables

    real = get_activation_tables  # functools.cache'd original

    def patched(arch):
        tabs = real(arch)
        return {
            name: (funcs if name == _ACT_SET else set())
            for name, funcs in tabs.items()
        }

    bacc_mod.get_activation_tables = patched


def _build_nc():
    AluOp = mybir.AluOpType
    ACT = mybir.ActivationFunctionType
    AX = mybir.AxisListType

    _pin_act_tables()
    nc = bacc.Bacc("TRN2", target_bir_lowering=False, debug=False, num_devices=M)

    xz_d = nc.dram_tensor("xz", [BL, XZ], BF16, kind="ExternalInput")
    xh_d = nc.dram_tensor("xh", [BL, XD], BF16, kind="ExternalInput")
    jq_d = nc.dram_tensor("jq", [BL, JQ], FP8, kind="ExternalInput")
    cst_d = nc.dram_tensor("cst", [P, NT], BF16, kind="ExternalInput")
    out_d = nc.dram_tensor("out", [1, 2], F32, kind="ExternalOutput")

    with tile.TileContext(nc) as tc, ExitStack() as ctx:
        cpool = ctx.enter_context(tc.tile_pool(name="consts", bufs=1))
        apool = ctx.enter_context(tc.tile_pool(name="a", bufs=4))
        bpool = ctx.enter_context(tc.tile_pool(name="b", bufs=3))
        spool = ctx.enter_context(tc.tile_pool(name="s", bufs=2))
        accp = ctx.enter_context(tc.tile_pool(name="acc", bufs=1))
        psp = ctx.enter_context(
            tc.tile_pool(name="ps", bufs=1, space=bass.MemorySpace.PSUM)
        )

        ybuf = cpool.tile([P, NT], BF16)        # y at column slot t
        ones = cpool.tile([P, 1], F32)

        ssqb = accp.tile([P, NT], F32)          # |dx|^2 + |z|^2 per row
        sjpb = accp.tile([P, NT], F32)          # sum_c exp(joint logit)
        jselb = accp.tile([P, NT], F32)         # joint logit at y
        s3b = accp.tile([P, NT, D], F32)        # per-dim sum_k exp(pred logit)
        nlselb = accp.tile([P, NT], F32)        # -sum_d pred logit at valid_cp[y]

        fb = accp.tile([P, 2, NT], F32)
        nc.gpsimd.dma_start(out=ybuf[:], in_=cst_d[:, 0:NT])
        nc.vector.memset(ones[:], 1.0)
        nc.vector.tensor_scalar(
            out=fb[:, 1, :], in0=ybuf[:], scalar1=float(C), scalar2=None,
            op0=AluOp.is_lt,
        )

        def emit_iter(i, ndve):
            r = slice(i * RA, (i + 1) * RA)
            cols = slice(i * S, (i + 1) * S)
            first = i == 0
            ct = apool.tile([P, S, XZ], BF16, tag="ct")
            xh = apool.tile([P, S, XD], BF16, tag="xh")
            jt = bpool.tile([P, S, JQ], FP8, tag="jt")
            hs = S // 2
            xzv = xz_d[r, :].rearrange("(p s) d -> p s d", s=S)
            xhv = xh_d[r, :].rearrange("(p s) d -> p s d", s=S)
            nc.sync.dma_start(out=ct[:, 0:hs, :], in_=xzv[:, 0:hs, :])
            nc.sync.dma_start(out=ct[:, hs:S, :], in_=xzv[:, hs:S, :])
            if first:
                nc.scalar.dma_start(out=xh[:, 0:hs, :], in_=xhv[:, 0:hs, :])
                nc.scalar.dma_start(out=xh[:, hs:S, :], in_=xhv[:, hs:S, :])
            else:
                nc.scalar.dma_start(out=xh[:], in_=xhv)
            nc.gpsimd.dma_start(
                out=jt[:], in_=jq_d[r, :].rearrange("(p s) d -> p s d", s=S)
            )

            # --- phase A: dx = x - xh (in place), then ssq per row ---
            nds = S - N_GPS_SUB  # slices subtracted on DVE
            if first:
                # halved ops so compute starts as soon as half the tile lands
                hd = nds // 2
                nc.vector.tensor_tensor(
                    out=ct[:, 0:hd, 0:XD], in0=ct[:, 0:hd, 0:XD],
                    in1=xh[:, 0:hd, :], op=AluOp.subtract,
                )
                nc.vector.tensor_tensor(
                    out=ct[:, hd:nds, 0:XD], in0=ct[:, hd:nds, 0:XD],
                    in1=xh[:, hd:nds, :], op=AluOp.subtract,
                )
            else:
                nc.vector.tensor_tensor(
                    out=ct[:, 0:nds, 0:XD], in0=ct[:, 0:nds, 0:XD],
                    in1=xh[:, 0:nds, :], op=AluOp.subtract,
                )
            nc.gpsimd.tensor_tensor(
                out=ct[:, nds:S, 0:XD], in0=ct[:, nds:S, 0:XD],
                in1=xh[:, nds:S, :], op=AluOp.subtract,
            )
            if ndve > 0:
                sq = spool.tile([P, ndve, XZ], BF16, tag="sq")
                nc.vector.tensor_tensor(
                    out=sq[:], in0=ct[:, 0:ndve, :], in1=ct[:, 0:ndve, :],
                    op=AluOp.mult,
                )
                nc.vector.tensor_reduce(
                    out=ssqb[:, i * S:i * S + ndve], in_=sq[:],
                    axis=AX.X, op=AluOp.add,
                )
            for s in range(ndve, S):
                t = i * S + s
                o = spool.tile([P, XZ], BF16, tag="acto")
                nc.scalar.activation(
                    out=o[:], in_=ct[:, s, :], func=ACT.Square,
                    accum_out=ssqb[:, t:t + 1],
                )

            # --- phase B: joint + pred log-softmax pieces ---
            # jq row: [jl (64) | pl (30) | ohj (64) | -ohp (30)]
            HQ = C + D * K                                      # 94
            ef = bpool.tile([P, S, HQ], F32, tag="ef")
            nc.scalar.activation(out=ef[:], in_=jt[:, :, 0:HQ], func=ACT.Exp)
            nc.vector.tensor_reduce(
                out=sjpb[:, cols], in_=ef[:, :, 0:C], axis=AX.X, op=AluOp.add
            )
            nc.vector.tensor_reduce(
                out=s3b[:, cols, :],
                in_=ef[:, :, C:HQ].rearrange("p s (d k) -> p s d k", k=K),
                axis=AX.X, op=AluOp.add,
            )
            # one-hot select with a single GpSimd multiply over [jl|pl]
            # (the pred one-hot is shipped negated, so the [64:94] sum
            # yields -sum_d lsel_d)
            ohm = bpool.tile([P, S, HQ], BF16, tag="ohm")
            nc.gpsimd.tensor_tensor(
                out=ohm[:], in0=jt[:, :, HQ:JQ], in1=jt[:, :, 0:HQ],
                op=AluOp.mult,
            )
            nc.vector.tensor_reduce(
                out=jselb[:, cols], in_=ohm[:, :, 0:C], axis=AX.X, op=AluOp.add
            )
            nc.vector.tensor_reduce(
                out=nlselb[:, cols], in_=ohm[:, :, C:HQ], axis=AX.X,
                op=AluOp.add,
            )

        # --- epilogue, emitted in two column halves so most of the final
        # pointwise math overlaps the last iteration's streaming ---
        lnsjp = accp.tile([P, NT], F32)
        s3p = accp.tile([P, NT], F32)
        lns3p = accp.tile([P, NT], F32)
        jd = accp.tile([P, NT], F32)
        t2 = accp.tile([P, NT], F32)
        diff = accp.tile([P, NT], F32)
        jps = accp.tile([P, NT], F32)
        wv = accp.tile([P, NT], F32)
        rr = accp.tile([P, 2, 2], F32)
        ps = psp.tile([1, 2], F32)
        osb = accp.tile([1, 2], F32)

        def emit_epi(c0, c1, part):
            cs = slice(c0, c1)
            nc.scalar.activation(out=lnsjp[:, cs], in_=sjpb[:, cs], func=ACT.Ln)
            nc.vector.tensor_tensor(
                out=s3p[:, cs], in0=s3b[:, cs, 0], in1=s3b[:, cs, 1],
                op=AluOp.mult,
            )
            nc.vector.tensor_tensor(
                out=s3p[:, cs], in0=s3p[:, cs], in1=s3b[:, cs, 2],
                op=AluOp.mult,
            )
            nc.scalar.activation(out=lns3p[:, cs], in_=s3p[:, cs], func=ACT.Ln)
            # lnjp = jsel - lnsjp ; lp = lsel - lns3p ; diff = lnjp - lp
            nc.vector.tensor_tensor(out=jd[:, cs], in0=jselb[:, cs],
                                    in1=lnsjp[:, cs], op=AluOp.subtract)
            nc.vector.tensor_tensor(out=t2[:, cs], in0=lns3p[:, cs],
                                    in1=nlselb[:, cs], op=AluOp.add)
            nc.vector.tensor_tensor(out=diff[:, cs], in0=jd[:, cs],
                                    in1=t2[:, cs], op=AluOp.add)
            nc.scalar.activation(out=jps[:, cs], in_=jd[:, cs], func=ACT.Exp)
            nc.scalar.activation(out=wv[:, cs], in_=ssqb[:, cs], func=ACT.Exp,
                                 scale=-0.5)
            nc.vector.tensor_tensor(out=diff[:, cs], in0=diff[:, cs],
                                    in1=jps[:, cs], op=AluOp.mult)
            nc.vector.tensor_tensor(out=diff[:, cs], in0=diff[:, cs],
                                    in1=wv[:, cs], op=AluOp.mult)
            nc.vector.tensor_tensor(out=fb[:, 0, cs], in0=diff[:, cs],
                                    in1=fb[:, 1, cs], op=AluOp.mult)
            nc.vector.tensor_reduce(out=rr[:, part, :], in_=fb[:, :, cs],
                                    axis=AX.X, op=AluOp.add)

        sched = DVE_SQ_SCHED or [N_DVE_SQ] * NA
        CE = (NA - 1) * S  # columns complete once iters 0..NA-2 are done
        for i in range(NA):
            emit_iter(i, sched[i])
            if i == NA - 2:
                emit_epi(0, CE, 0)
        emit_epi(CE, NT, 1)
        nc.tensor.matmul(ps[:], ones[:], rr[:, 0, :], start=True, stop=False)
        nc.tensor.matmul(ps[:], ones[:], rr[:, 1, :], start=False, stop=True)
        nc.vector.tensor_copy(out=osb[:], in_=ps[:])
        nc.sync.dma_start(out=out_d[:], in_=osb[:])

    nc.compile()
    return nc


def _get_nc():
    if "nc" not in _NC_CACHE:
        _NC_CACHE["nc"] = _build_nc()
    return _NC_CACHE["nc"]


def _col_layout(arr):
    """[BL, ...] per-core rows -> [P, NT, ...] SBUF column layout where row
    i*RA + p*S + s lands at [p, i*S + s]."""
    tail = arr.shape[1:]
    a = arr.reshape(NA, P, S, *tail)          # [i, p, s, ...]
    a = np.moveaxis(a, 1, 0)                  # [p, i, s, ...]
    return np.ascontiguousarray(a.reshape(P, NT, *tail))


def _prep_in_maps(inputs):
    bf16 = ml_dtypes.bfloat16
    X = np.asarray(inputs["X"], dtype=np.float32)
    Z = np.asarray(inputs["Z"], dtype=np.float32)
    XZc = np.concatenate([X, Z], axis=1).astype(bf16)
    Xh = np.asarray(inputs["X_hat"], dtype=np.float32).astype(bf16)
    jp64 = np.asarray(inputs["joint_probs"], dtype=np.float32)[:, :C]
    ppf = (
        np.asarray(inputs["pred_probs"], dtype=np.float32)
        .transpose(1, 0, 2)
        .reshape(B, D * K)
    )
    y = np.asarray(inputs["Y_valid"])
    vcp = np.asarray(inputs["valid_cp"])
    y_safe = np.where(y < C, y, 0).astype(np.int64)
    v3 = vcp[y_safe]                          # [B, 3]
    bidx = np.arange(B)
    ohj = np.zeros((B, C), np.float32)
    ohj[bidx, y_safe] = 1.0
    ohp = np.zeros((B, D, K), np.float32)
    for d in range(D):
        ohp[bidx, d, v3[:, d]] = -1.0          # negated: [64:94] sum = -lsel
    JQc = np.concatenate(
        [jp64, ppf, ohj, ohp.reshape(B, D * K)], axis=1
    ).astype(ml_dtypes.float8_e4m3)
    y32 = y.astype(np.float32)

    in_maps = []
    for m in range(M):
        s = slice(m * BL, (m + 1) * BL)
        in_maps.append(
            {
                "xz": np.ascontiguousarray(XZc[s]),
                "xh": np.ascontiguousarray(Xh[s]),
                "jq": np.ascontiguousarray(JQc[s]),
                "cst": _col_layout(y32[s]).astype(bf16),
            }
        )
    return in_maps


def _combine(results):
    tot = 0.0
    cnt = 0.0
    for r in results:
        o = np.asarray(r["out"], dtype=np.float64)
        tot += float(o[0, 0])
        cnt += float(o[0, 1])
    loss = abs(tot)
    val = loss / cnt if cnt > 0 else loss
    return np.float32(val)


def run(inputs, trace=False, **kwargs):
    """Build (cached), run on the 8 NeuronCores, return (value, BassKernelResults)."""
    nc = _get_nc()
    in_maps = _prep_in_maps(inputs)
    res = run_bass_kernel_spmd(nc, in_maps, list(range(M)), trace=trace, **kwargs)
    return _combine(res.results), res


def kernel(**inputs):
    val, _ = run(inputs, trace=False)
    return val
